# revision 1
# baseline (speedup 1.0000x reference)
"""LLaDA2 MoE decoder layer on 8 TRN2 NeuronCores.

Token-sharded attention (each core: all 16 heads for its 128 tokens, kv
projection replicated), one AllGather of post-attention normed hidden
(transposed layout), expert-parallel dense MoE (2 experts/core, gate
columns permuted per-core so local experts are columns 0,1), shared
expert token-sharded. Host sums the 8 partial outputs.
"""
import numpy as np
import concourse.bass as bass
import concourse.bacc as bacc
import concourse.mybir as mybir
import concourse.tile as tile
from concourse.bass_utils import run_bass_kernel_spmd

AF = mybir.ActivationFunctionType
ALU = mybir.AluOpType
F32 = mybir.dt.float32
F32R = mybir.dt.float32r
BF16 = mybir.dt.bfloat16

B, S, H = 1, 1024, 2048
NH, HD, NKV, ROT = 16, 128, 4, 64
E, TOPK, G = 16, 4, 2
MI = 1024
T = S
NCORES = 8
TL = T // NCORES
SCAL = HD ** -0.5
EPS = 1e-6
NDH = H // 128
NMI = MI // 128

_BUILT = {}


def _spec():
    return [
        ("hidT", [H, T], F32), ("hidTl", [H, TL], F32),
        ("onec", [128, 1], F32R),
        ("cosl", [ROT, TL], F32), ("sinl", [ROT, TL], F32),
        ("cosf", [ROT, T], F32), ("sinf", [ROT, T], F32),
        ("qln", [HD, 1], F32), ("kln", [HD, 1], F32),
        ("ln1c", [H, 1], F32), ("ln2c", [H, 1], F32),
        ("wqT", [H, NH * HD], BF16), ("wkT", [H, NKV * HD], BF16),
        ("wvT", [H, NKV * HD], BF16), ("wdT", [NH * HD, H], BF16),
        ("gT", [H, E], F32R), ("eb", [1, E], F32),
        ("g0T", [H, MI], BF16), ("u0T", [H, MI], BF16), ("d0T", [MI, H], BF16),
        ("g1T", [H, MI], BF16), ("u1T", [H, MI], BF16), ("d1T", [MI, H], BF16),
        ("sgT", [H, MI], BF16), ("suT", [H, MI], BF16), ("sdT", [MI, H], BF16),
    ]


def _build():
    if "nc" in _BUILT:
        return _BUILT["nc"]
    nc = bacc.Bacc("TRN2", target_bir_lowering=False, debug=False,
                   num_devices=NCORES)
    I = {}
    for name, shp, dt in _spec():
        I[name] = nc.dram_tensor(name, shp, dt, kind="ExternalInput")
    routed = nc.dram_tensor("routed", [T, H], F32, kind="ExternalOutput")
    own = nc.dram_tensor("own", [TL, H], F32, kind="ExternalOutput")
    xout = nc.dram_tensor("xout", [H, TL], F32, kind="ExternalOutput")

    with tile.TileContext(nc) as tc, \
         tc.tile_pool(name="cst", bufs=1) as cst, \
         tc.tile_pool(name="big", bufs=16) as big, \
         tc.tile_pool(name="kro", bufs=4) as krop, \
         tc.tile_pool(name="vp", bufs=8) as vp, \
         tc.tile_pool(name="otp", bufs=16) as otp, \
         tc.tile_pool(name="agl", bufs=16) as agl, \
         tc.tile_pool(name="a12", bufs=16) as a12p, \
         tc.tile_pool(name="wrk", bufs=2) as wrk, \
         tc.tile_pool(name="w128", bufs=8) as w128, \
         tc.tile_pool(name="w512", bufs=2) as w512, \
         tc.tile_pool(name="yp", bufs=2) as yp, \
         tc.tile_pool(name="psA", bufs=4, space="PSUM") as psA, \
         tc.tile_pool(name="psB", bufs=4, space="PSUM") as psB, \
         tc.tile_pool(name="dram", bufs=1, space="DRAM") as dpool:

        ones = cst.tile([128, 1], F32R, tag="ones")
        nc.sync.dma_start(out=ones[:, :], in_=I["onec"][:, :])
        ones_bf = cst.tile([128, 1], BF16, tag="ones_bf")
        nc.vector.memset(ones_bf[:, :], 1.0)
        epsA = cst.tile([128, 1], F32, tag="epsA")
        nc.vector.memset(epsA[:, :], EPS)
        invH = cst.tile([128, 1], F32, tag="invH")
        nc.vector.memset(invH[:, :], 1.0 / H)
        invHD = cst.tile([128, 1], F32, tag="invHD")
        nc.vector.memset(invHD[:, :], 1.0 / HD)
        scalA = cst.tile([128, 1], F32, tag="scalA")
        nc.vector.memset(scalA[:, :], SCAL)

        def cload(name, shp, key):
            t_ = cst.tile(shp, F32, tag=key)
            nc.sync.dma_start(out=t_[:, :], in_=I[name][:, :])
            return t_
        qln = cload("qln", [HD, 1], "qln")
        kln = cload("kln", [HD, 1], "kln")
        cosl = cload("cosl", [ROT, TL], "cosl")
        sinl = cload("sinl", [ROT, TL], "sinl")
        cosf = cload("cosf", [ROT, T], "cosf")
        sinf = cload("sinf", [ROT, T], "sinf")
        ebbc = cst.tile([128, E], F32, tag="ebbc")
        nc.sync.dma_start(out=ebbc[:, :],
                          in_=I["eb"][0:1, :].partition_broadcast(128))

        def bcast(row_ap, n, tag, out_tile):
            d_ = dpool.tile([1, n], F32, tag=tag + "_d", bufs=2,
                            name=tag + "_d")
            nc.sync.dma_start(out=d_[0:1, :], in_=row_ap)
            nc.sync.dma_start(out=out_tile[:, :],
                              in_=d_[0:1, :].partition_broadcast(128))

        # ---- r_row over H from hidT (streamed) ----
        ssq = [psB.tile([1, 512], F32, tag="psB", name=f"ssq{c}")
               for c in range(2)]
        for i in range(NDH):
            ht = wrk.tile([128, T], F32, tag="hidT", bufs=2)
            nc.sync.dma_start(out=ht[:, :], in_=I["hidT"][i * 128:(i + 1) * 128, :])
            sq = wrk.tile([128, T], F32R, tag="sq", bufs=2)
            nc.scalar.activation(sq[:, :], ht[:, :], AF.Square)
            for c in range(2):
                nc.tensor.matmul(ssq[c][:, :], ones[:, :],
                                 sq[:, c * 512:(c + 1) * 512],
                                 start=(i == 0), stop=(i == NDH - 1))
        r_row = wrk.tile([1, T], F32, tag="rrow", bufs=1)
        rsq = wrk.tile([1, T], F32, tag="rsq", bufs=1)
        for c in range(2):
            nc.scalar.activation(rsq[0:1, c * 512:(c + 1) * 512], ssq[c][:, :],
                                 AF.Sqrt, bias=epsA[0:1, 0:1],
                                 scale=invH[0:1, 0:1])
        nc.vector.reciprocal(r_row[0:1, :], rsq[0:1, :])
        rbc = wrk.tile([128, T], F32, tag="rbc", bufs=1)
        bcast(r_row[0:1, :], T, "rbc", rbc)

        # ---- xnT = hidT * ln1 * r (transposed normed hidden, f32r) ----
        xnT = []
        for i in range(NDH):
            ht = wrk.tile([128, T], F32, tag="hidT", bufs=2)
            nc.sync.dma_start(out=ht[:, :], in_=I["hidT"][i * 128:(i + 1) * 128, :])
            lnc = wrk.tile([128, 1], F32, tag="lnc", bufs=2)
            nc.sync.dma_start(out=lnc[:, :], in_=I["ln1c"][i * 128:(i + 1) * 128, :])
            xt = big.tile([128, T], BF16, tag="big")
            nc.vector.scalar_tensor_tensor(xt[:, :], ht[:, :], lnc[:, 0:1],
                                           rbc[:, :], ALU.mult, ALU.mult)
            xnT.append(xt)

        # ---- local-token normed tiles for q projection ----
        ssl = psB.tile([1, TL], F32, tag="psB", name="ssl")
        for i in range(NDH):
            htl = wrk.tile([128, TL], F32, tag="htl", bufs=2)
            nc.sync.dma_start(out=htl[:, :], in_=I["hidTl"][i * 128:(i + 1) * 128, :])
            sql = wrk.tile([128, TL], F32R, tag="sql", bufs=2)
            nc.scalar.activation(sql[:, :], htl[:, :], AF.Square)
            nc.tensor.matmul(ssl[:, :], ones[:, :], sql[:, :],
                             start=(i == 0), stop=(i == NDH - 1))
        rls = wrk.tile([1, TL], F32, tag="rls", bufs=1)
        nc.scalar.activation(rls[0:1, :], ssl[:, :], AF.Sqrt,
                             bias=epsA[0:1, 0:1], scale=invH[0:1, 0:1])
        rl = wrk.tile([1, TL], F32, tag="rl", bufs=1)
        nc.vector.reciprocal(rl[0:1, :], rls[0:1, :])
        rlb = wrk.tile([128, TL], F32, tag="rlb", bufs=1)
        bcast(rl[0:1, :], TL, "rlb", rlb)
        xnTl = []
        for i in range(NDH):
            htl2 = wrk.tile([128, TL], F32, tag="htl", bufs=2)
            nc.sync.dma_start(out=htl2[:, :],
                              in_=I["hidTl"][i * 128:(i + 1) * 128, :])
            lnc2 = wrk.tile([128, 1], F32, tag="lnc", bufs=2)
            nc.sync.dma_start(out=lnc2[:, :], in_=I["ln1c"][i * 128:(i + 1) * 128, :])
            xl = wrk.tile([128, TL], BF16, tag="xnTl", bufs=16)
            nc.vector.scalar_tensor_tensor(xl[:, :], htl2[:, :], lnc2[:, 0:1],
                                           rlb[:, :], ALU.mult, ALU.mult)
            xnTl.append(xl)

        def rms_cols(ps, n, lnw, out_ap):
            """out = ps * lnw * rsqrt(mean_part(ps^2)+eps); ps [128,n] psum."""
            sqk = wrk.tile([128, n], F32R, tag="sqk", bufs=1)
            nc.scalar.activation(sqk[:, :], ps[:, :], AF.Square)
            ssk = psB.tile([1, n], F32, tag="psB")
            nc.tensor.matmul(ssk[:, :], ones[:, :], sqk[:, :], start=True, stop=True)
            rks = wrk.tile([1, n], F32, tag="rks", bufs=1)
            nc.scalar.activation(rks[0:1, :], ssk[:, :], AF.Sqrt,
                                 bias=epsA[0:1, 0:1], scale=invHD[0:1, 0:1])
            rk = wrk.tile([1, n], F32, tag="rk", bufs=1)
            nc.vector.reciprocal(rk[0:1, :], rks[0:1, :])
            rkb = wrk.tile([128, n], F32, tag="rkb", bufs=1)
            bcast(rk[0:1, :], n, "rkb", rkb)
            nc.vector.scalar_tensor_tensor(out_ap, ps[:, :], lnw[:, 0:1],
                                           rkb[:, :], ALU.mult, ALU.mult)

        def rope(dst, src, cos_t, sin_t, n):
            """dst[0:128,n] f32r from src f32: rows 0..63 roped, 64..127 copy."""
            nc.vector.tensor_copy(dst[ROT:HD, :], src[ROT:HD, :])
            sh = wrk.tile([ROT, n], F32, tag="sh", bufs=1)
            nc.sync.dma_start(out=sh[0:32, :], in_=src[32:64, :])
            nc.sync.dma_start(out=sh[32:64, :], in_=src[0:32, :])
            tm = wrk.tile([ROT, n], F32, tag="tm", bufs=1)
            nc.vector.tensor_tensor(tm[:, :], src[0:ROT, :], cos_t[:, :], ALU.mult)
            tm2 = wrk.tile([ROT, n], F32, tag="tm2", bufs=1)
            nc.vector.tensor_tensor(tm2[:, :], sh[:, :], sin_t[:, :], ALU.mult)
            nc.vector.tensor_tensor(dst[0:ROT, :], tm[:, :], tm2[:, :], ALU.add)

        # ---- k heads: project, rms, rope -> kro[g] [128, T] f32r ----
        kro = []
        for g in range(NKV):
            kr = krop.tile([128, T], BF16, tag="kro")
            for c in range(2):
                sl = slice(c * 512, (c + 1) * 512)
                ps = psA.tile([128, 512], F32, tag="psA")
                for i in range(NDH):
                    wt = w128.tile([128, 128], BF16, tag="w128")
                    nc.sync.dma_start(
                        out=wt[:, :],
                        in_=I["wkT"][i * 128:(i + 1) * 128, g * 128:(g + 1) * 128])
                    nc.tensor.matmul(ps[:, :], wt[:, :], xnT[i][:, sl],
                                     start=(i == 0), stop=(i == NDH - 1))
                kf = wrk.tile([128, 512], F32, tag="kf", bufs=2)
                rms_cols(ps, 512, kln, kf[:, :])
                rope(kr[:, sl], kf, cosf[:, sl], sinf[:, sl], 512)
            kro.append(kr)

        # ---- v token-major [t-tile, 512] f32r ----
        vsb = []
        for j in range(8):
            ps = psA.tile([128, 512], F32, tag="psA")
            for i in range(NDH):
                wt = w512.tile([128, 512], BF16, tag="w512")
                nc.sync.dma_start(out=wt[:, :],
                                  in_=I["wvT"][i * 128:(i + 1) * 128, :])
                nc.tensor.matmul(ps[:, :], xnT[i][:, j * 128:(j + 1) * 128],
                                 wt[:, :], start=(i == 0), stop=(i == NDH - 1))
            vt = vp.tile([128, 512], BF16, tag="vp")
            nc.vector.tensor_copy(vt[:, :], ps[:, :])
            vsb.append(vt)

        # ---- per q-head: project(local), rms, rope, scores, probs, oT ----
        oT = []
        for h in range(NH):
            g = h // (NH // NKV)
            ps = psB.tile([128, TL], F32, tag="psB")
            for i in range(NDH):
                wt = w128.tile([128, 128], BF16, tag="w128")
                nc.sync.dma_start(
                    out=wt[:, :],
                    in_=I["wqT"][i * 128:(i + 1) * 128, h * 128:(h + 1) * 128])
                nc.tensor.matmul(ps[:, :], wt[:, :], xnTl[i][:, :],
                                 start=(i == 0), stop=(i == NDH - 1))
            qf = wrk.tile([128, TL], F32, tag="qf", bufs=2)
            rms_cols(ps, TL, qln, qf[:, :])
            qr = wrk.tile([128, TL], BF16, tag="qr", bufs=2)
            rope(qr, qf, cosl, sinl, TL)
            # scores^T tiles [tk 128, tq 128]; probs = exp(s*SCAL); oT accum
            pso = psB.tile([128, TL], F32, tag="psB")
            psz = psB.tile([1, TL], F32, tag="psB")
            for tk in range(8):
                sps = psB.tile([128, TL], F32, tag="psB")
                nc.tensor.matmul(sps[:, :], kro[g][:, tk * 128:(tk + 1) * 128],
                                 qr[:, :], start=True, stop=True)
                pr = wrk.tile([128, TL], BF16, tag="pr", bufs=3)
                nc.scalar.activation(pr[:, :], sps[:, :], AF.Exp,
                                     scale=scalA[:, 0:1])
                nc.tensor.matmul(pso[:, :], vsb[tk][:, g * 128:(g + 1) * 128],
                                 pr[:, :], start=(tk == 0), stop=(tk == 7))
                nc.tensor.matmul(psz[:, :], ones_bf[:, :], pr[:, :],
                                 start=(tk == 0), stop=(tk == 7))
            zr = wrk.tile([1, TL], F32, tag="zr", bufs=2)
            nc.vector.reciprocal(zr[0:1, :], psz[:, :])
            zbc = wrk.tile([128, TL], F32, tag="zbc", bufs=2)
            bcast(zr[0:1, :], TL, "zbc", zbc)
            ot = otp.tile([128, TL], BF16, tag="oT")
            nc.vector.tensor_tensor(ot[:, :], pso[:, :], zbc[:, :], ALU.mult)
            oT.append(ot)

        # ---- attn_outT + residual -> xT; rms -> hT (f32r) + xout/ag_in ----
        ag_in = dpool.tile([H, TL], F32R, tag="agin")
        ag_out = dpool.tile([NCORES * H, TL], F32R, tag="agout",
                            addr_space="Shared")
        hT_l = []
        for i in range(NDH):
            ps = psB.tile([128, TL], F32, tag="psB")
            for d in range(NH):
                wt = w128.tile([128, 128], BF16, tag="w128")
                nc.sync.dma_start(
                    out=wt[:, :],
                    in_=I["wdT"][d * 128:(d + 1) * 128, i * 128:(i + 1) * 128])
                nc.tensor.matmul(ps[:, :], wt[:, :], oT[d][:, :],
                                 start=(d == 0), stop=(d == NH - 1))
            hl = wrk.tile([128, TL], F32, tag="hl", bufs=2)
            nc.sync.dma_start(out=hl[:, :], in_=I["hidTl"][i * 128:(i + 1) * 128, :])
            xt = agl.tile([128, TL], F32, tag="xT")
            nc.vector.tensor_tensor(xt[:, :], ps[:, :], hl[:, :], ALU.add)
            nc.sync.dma_start(out=xout[i * 128:(i + 1) * 128, :], in_=xt[:, :])
            hT_l.append(xt)
        # second rms (over H, partition dim) via ones-matmul on squares
        ss2 = psB.tile([1, TL], F32, tag="psB")
        sq2t = []
        for i in range(NDH):
            s2 = wrk.tile([128, TL], F32R, tag="s2", bufs=16)
            nc.scalar.activation(s2[:, :], hT_l[i][:, :], AF.Square)
            sq2t.append(s2)
        for i in range(NDH):
            nc.tensor.matmul(ss2[:, :], ones[:, :], sq2t[i][:, :],
                             start=(i == 0), stop=(i == NDH - 1))
        r2s = wrk.tile([1, TL], F32, tag="r2s", bufs=1)
        nc.scalar.activation(r2s[0:1, :], ss2[:, :], AF.Sqrt,
                             bias=epsA[0:1, 0:1], scale=invH[0:1, 0:1])
        r2 = wrk.tile([1, TL], F32, tag="r2", bufs=1)
        nc.vector.reciprocal(r2[0:1, :], r2s[0:1, :])
        r2b = wrk.tile([128, TL], F32, tag="r2b", bufs=1)
        bcast(r2[0:1, :], TL, "r2b", r2b)
        hTt = []
        for i in range(NDH):
            ln2 = wrk.tile([128, 1], F32, tag="ln2", bufs=2)
            nc.sync.dma_start(out=ln2[:, :], in_=I["ln2c"][i * 128:(i + 1) * 128, :])
            ht = agl.tile([128, TL], F32R, tag="hTl")
            nc.vector.scalar_tensor_tensor(ht[:, :], hT_l[i][:, :], ln2[:, 0:1],
                                           r2b[:, :], ALU.mult, ALU.mult)
            nc.sync.dma_start(out=ag_in[i * 128:(i + 1) * 128, :], in_=ht[:, :])
            hTt.append(ht)

        nc.gpsimd.collective_compute(
            "AllGather", ALU.bypass, ins=[ag_in], outs=[ag_out],
            replica_groups=[list(range(NCORES))])

        # ---- load gathered hT [2048, 1024] f32r into big pool ----
        agv = ag_out.rearrange("(b d) t -> d b t", b=NCORES)
        hsb = []
        for i in range(NDH):
            t_ = big.tile([128, T], BF16, tag="big")
            nc.gpsimd.dma_start(out=t_[:, :], in_=agv[i * 128:(i + 1) * 128, :, :])
            hsb.append(t_)
        hbf = []
        for i in range(NDH):
            hb = agl.tile([128, TL], BF16, tag="hbf")
            nc.vector.tensor_copy(hb[:, :], hTt[i][:, :])
            hbf.append(hb)
        _BUILT["ctx"] = dict(nc=nc, tc=tc, I=I, routed=routed, own=own,
                             hsb=hsb, hTt=hTt, ones=ones, ebbc=ebbc,
                             pools=dict(cst=cst, a12p=a12p, wrk=wrk, w128=w128,
                                        w512=w512, yp=yp, psA=psA, psB=psB))
        _moe(nc, tc, I, routed, own, hsb, hbf, agv, ebbc,
             a12p, wrk, w128, w512, yp, psA, psB)
    nc.compile()
    _BUILT["nc"] = nc
    return nc


def _moe(nc, tc, I, routed, own, hsb, hbf, agv, ebbc, a12p, wrk, w128, w512,
         yp, psA, psB):
    # ---- routing (replicated, all tokens): we [128,16] f32 per t-tile ----
    gts = []
    for i in range(NDH):
        gt = wrk.tile([128, E], F32R, tag="gt", bufs=16)
        nc.sync.dma_start(out=gt[:, :], in_=I["gT"][i * 128:(i + 1) * 128, :])
        gts.append(gt)
    we_sb = []
    for j in range(8):
        pl = psB.tile([128, E], F32, tag="psB")
        for i in range(NDH):
            hl_ = wrk.tile([128, 128], F32R, tag="hload", bufs=3)
            nc.sync.dma_start(out=hl_[:, :],
                              in_=agv[i * 128:(i + 1) * 128, j, :])
            nc.tensor.matmul(pl[:, :], hl_[:, :], gts[i][:, :],
                             start=(i == 0), stop=(i == NDH - 1))
        s = wrk.tile([128, E], F32, tag="rs", bufs=2)
        nc.scalar.activation(s[:, :], pl[:, :], AF.Sigmoid)
        sfr = wrk.tile([128, E], F32, tag="sfr", bufs=2)
        nc.vector.tensor_tensor(sfr[:, :], s[:, :], ebbc[:, :], ALU.add)
        msk = wrk.tile([128, E], F32, tag="msk", bufs=2)
        m1 = wrk.tile([128, 2], F32, tag="m1", bufs=2)
        m2 = wrk.tile([128, 2], F32, tag="m2", bufs=2)
        tmp8 = wrk.tile([128, 8], F32, tag="tmp8", bufs=2)
        for g in range(2):
            hv = sfr[:, g * 8:(g + 1) * 8]
            nc.vector.tensor_reduce(m1[:, g:g + 1], hv, mybir.AxisListType.X,
                                    ALU.max)
            eq = wrk.tile([128, 8], F32, tag="eq", bufs=2)
            nc.vector.tensor_scalar(eq[:, :], hv, m1[:, g:g + 1], None,
                                    ALU.is_equal)
            nc.vector.scalar_tensor_tensor(tmp8[:, :], eq[:, :], -1e30,
                                           hv, ALU.mult, ALU.add)
            nc.vector.tensor_reduce(m2[:, g:g + 1], tmp8[:, :],
                                    mybir.AxisListType.X, ALU.max)
        gs = wrk.tile([128, 2], F32, tag="gs", bufs=2)
        nc.vector.tensor_tensor(gs[:, :], m1[:, :], m2[:, :], ALU.add)
        gd = wrk.tile([128, 1], F32, tag="gd", bufs=2)
        nc.vector.tensor_tensor(gd[:, :], gs[:, 0:1], gs[:, 1:2], ALU.subtract)
        ka = wrk.tile([128, 2], F32, tag="ka", bufs=2)
        nc.vector.tensor_scalar(ka[:, 0:1], gd[:, :], 0.0, None, ALU.is_ge)
        nc.vector.tensor_scalar(ka[:, 1:2], ka[:, 0:1], -1.0, 1.0,
                                ALU.mult, ALU.add)
        for g in range(2):
            nc.vector.tensor_scalar(msk[:, g * 8:(g + 1) * 8],
                                    sfr[:, g * 8:(g + 1) * 8],
                                    ka[:, g:g + 1], None, ALU.mult)
        # 4th-largest threshold of msk
        w0 = wrk.tile([128, E], F32, tag="w0", bufs=2)
        nc.vector.tensor_copy(w0[:, :], msk[:, :])
        tau = wrk.tile([128, 1], F32, tag="tau", bufs=2)
        lt = wrk.tile([128, E], F32, tag="lt", bufs=2)
        for it in range(3):
            nc.vector.tensor_reduce(tau[:, :], w0[:, :], mybir.AxisListType.X,
                                    ALU.max)
            nc.vector.tensor_scalar(lt[:, :], w0[:, :], tau[:, 0:1], None,
                                    ALU.is_lt)
            nc.vector.tensor_tensor(w0[:, :], w0[:, :], lt[:, :], ALU.mult)
        nc.vector.tensor_reduce(tau[:, :], w0[:, :], mybir.AxisListType.X,
                                ALU.max)
        sel = wrk.tile([128, E], F32, tag="sel", bufs=2)
        nc.vector.tensor_scalar(sel[:, :], msk[:, :], tau[:, 0:1], None,
                                ALU.is_ge)
        wsel = wrk.tile([128, E], F32, tag="wsel", bufs=2)
        nc.vector.tensor_tensor(wsel[:, :], s[:, :], sel[:, :], ALU.mult)
        dn = wrk.tile([128, 1], F32, tag="dn", bufs=2)
        nc.vector.tensor_reduce(dn[:, :], wsel[:, :], mybir.AxisListType.X,
                                ALU.add)
        nc.vector.tensor_scalar(dn[:, :], dn[:, :], 1e-20, None, ALU.add)
        rc = wrk.tile([128, 1], F32, tag="rc", bufs=2)
        nc.vector.reciprocal(rc[:, :], dn[:, :])
        we = wrk.tile([128, E], F32, tag="we", bufs=16)
        nc.vector.tensor_scalar(we[:, :], wsel[:, :], rc[:, 0:1], None,
                                ALU.mult)
        we_sb.append(we)

    # ---- routed experts: dense over all tokens, 2 local experts ----
    for tc_i in range(2):          # token chunk of 512
        tsl = slice(tc_i * 512, (tc_i + 1) * 512)
        a12 = {}
        for mi in range(NMI):
            pg0 = psA.tile([128, 512], F32, tag="psA")
            pu0 = psA.tile([128, 512], F32, tag="psA")
            pg1 = psA.tile([128, 512], F32, tag="psA")
            pu1 = psA.tile([128, 512], F32, tag="psA")
            for i in range(NDH):
                rh = hsb[i][:, tsl]
                for (wn, ps) in (("g0T", pg0), ("u0T", pu0),
                                 ("g1T", pg1), ("u1T", pu1)):
                    wt = w128.tile([128, 128], BF16, tag="w128")
                    nc.sync.dma_start(
                        out=wt[:, :],
                        in_=I[wn][i * 128:(i + 1) * 128, mi * 128:(mi + 1) * 128])
                    nc.tensor.matmul(ps[:, :], wt[:, :], rh,
                                     start=(i == 0), stop=(i == NDH - 1))
            for e, (pg, pu) in enumerate(((pg0, pu0), (pg1, pu1))):
                sg = wrk.tile([128, 512], F32, tag="sg", bufs=3)
                nc.scalar.activation(sg[:, :], pg[:, :], AF.Silu)
                at = a12p.tile([128, 512], BF16, tag="a12")
                nc.vector.tensor_tensor(at[:, :], sg[:, :], pu[:, :], ALU.mult)
                a12[(e, mi)] = at
        for ho in range(4):
            ed0 = []
            ed1 = []
            for mi in range(NMI):
                w0_ = w512.tile([128, 512], BF16, tag="edp0", bufs=8)
                nc.sync.dma_start(
                    out=w0_[:, :],
                    in_=I["d0T"][mi * 128:(mi + 1) * 128, ho * 512:(ho + 1) * 512])
                ed0.append(w0_)
                w1_ = w512.tile([128, 512], BF16, tag="edp1", bufs=8)
                nc.sync.dma_start(
                    out=w1_[:, :],
                    in_=I["d1T"][mi * 128:(mi + 1) * 128, ho * 512:(ho + 1) * 512])
                ed1.append(w1_)
            for ts in range(4):
                jj = tc_i * 4 + ts
                cs = slice(ts * 128, (ts + 1) * 128)
                p0 = psA.tile([128, 512], F32, tag="psA")
                for mi in range(NMI):
                    nc.tensor.matmul(p0[:, :], a12[(0, mi)][:, cs], ed0[mi][:, :],
                                     start=(mi == 0), stop=(mi == NMI - 1))
                y = yp.tile([128, 512], F32, tag="y")
                nc.vector.tensor_scalar(y[:, :], p0[:, :],
                                        we_sb[jj][:, 0:1], None, ALU.mult)
                p1 = psA.tile([128, 512], F32, tag="psA")
                for mi in range(NMI):
                    nc.tensor.matmul(p1[:, :], a12[(1, mi)][:, cs], ed1[mi][:, :],
                                     start=(mi == 0), stop=(mi == NMI - 1))
                nc.vector.scalar_tensor_tensor(y[:, :], p1[:, :],
                                               we_sb[jj][:, 1:2], y[:, :],
                                               ALU.mult, ALU.add)
                nc.sync.dma_start(
                    out=routed[jj * 128:(jj + 1) * 128, ho * 512:(ho + 1) * 512],
                    in_=y[:, :])

    # ---- shared expert on local 128 tokens ----
    a12s = []
    for mi in range(NMI):
        pg = psA.tile([128, TL], F32, tag="psA")
        pu = psA.tile([128, TL], F32, tag="psA")
        for i in range(NDH):
            for (wn, ps) in (("sgT", pg), ("suT", pu)):
                wt = w128.tile([128, 128], BF16, tag="w128")
                nc.sync.dma_start(
                    out=wt[:, :],
                    in_=I[wn][i * 128:(i + 1) * 128, mi * 128:(mi + 1) * 128])
                nc.tensor.matmul(ps[:, :], wt[:, :], hbf[i][:, :],
                                 start=(i == 0), stop=(i == NDH - 1))
        sg = wrk.tile([128, TL], F32, tag="sgs", bufs=2)
        nc.scalar.activation(sg[:, :], pg[:, :], AF.Silu)
        at = a12p.tile([128, TL], BF16, tag="a12s", bufs=8)
        nc.vector.tensor_tensor(at[:, :], sg[:, :], pu[:, :], ALU.mult)
        a12s.append(at)
    for ho in range(4):
        ps = psA.tile([128, 512], F32, tag="psA")
        for mi in range(NMI):
            wt = w512.tile([128, 512], BF16, tag="sdw", bufs=4)
            nc.sync.dma_start(
                out=wt[:, :],
                in_=I["sdT"][mi * 128:(mi + 1) * 128, ho * 512:(ho + 1) * 512])
            nc.tensor.matmul(ps[:, :], a12s[mi][:, :], wt[:, :],
                             start=(mi == 0), stop=(mi == NMI - 1))
        y = yp.tile([128, 512], F32, tag="y")
        nc.vector.tensor_copy(y[:, :], ps[:, :])
        nc.sync.dma_start(out=own[:, ho * 512:(ho + 1) * 512], in_=y[:, :])


def _prep(inputs):
    """Per-core input dicts from full inputs."""
    hs = np.ascontiguousarray(inputs["hidden_states"][0])      # [T, H]
    hidT = np.ascontiguousarray(hs.T)                          # [H, T]
    cos = np.ascontiguousarray(inputs["cos"][0].T)             # [ROT, T]
    sin = inputs["sin"][0].T                                   # [ROT, T]
    sgn = np.ones((ROT, 1), np.float32)
    sgn[:32] = -1.0
    sins = np.ascontiguousarray(sin * sgn)
    wqkv = inputs["w_qkv"]
    wqT = np.ascontiguousarray(wqkv[:NH * HD].T)
    wkT = np.ascontiguousarray(wqkv[NH * HD:NH * HD + NKV * HD].T)
    wvT = np.ascontiguousarray(wqkv[NH * HD + NKV * HD:].T)
    wdT = np.ascontiguousarray(inputs["w_dense"].T)
    maps = []
    for c in range(NCORES):
        glo = c // 4
        loc = [2 * c, 2 * c + 1]
        grp = [glo * 8 + k for k in range(8)]
        rest = [e for e in grp if e not in loc]
        other = [(1 - glo) * 8 + k for k in range(8)]
        perm = loc + rest + other
        m = dict(
            hidT=hidT, hidTl=np.ascontiguousarray(hidT[:, c * TL:(c + 1) * TL]),
            onec=np.ones((128, 1), np.float32),
            cosl=np.ascontiguousarray(cos[:, c * TL:(c + 1) * TL]),
            sinl=np.ascontiguousarray(sins[:, c * TL:(c + 1) * TL]),
            cosf=cos, sinf=sins,
            qln=np.ascontiguousarray(inputs["q_ln_w"][:, None]),
            kln=np.ascontiguousarray(inputs["k_ln_w"][:, None]),
            ln1c=np.ascontiguousarray(inputs["ln1_w"][:, None]),
            ln2c=np.ascontiguousarray(inputs["ln2_w"][:, None]),
            wqT=wqT, wkT=wkT, wvT=wvT, wdT=wdT,
            gT=np.ascontiguousarray(inputs["gate_w"][perm].T),
            eb=np.ascontiguousarray(inputs["expert_bias"][perm][None, :]),
            g0T=np.ascontiguousarray(inputs["eg"][loc[0]].T),
            u0T=np.ascontiguousarray(inputs["eu"][loc[0]].T),
            d0T=np.ascontiguousarray(inputs["ed"][loc[0]].T),
            g1T=np.ascontiguousarray(inputs["eg"][loc[1]].T),
            u1T=np.ascontiguousarray(inputs["eu"][loc[1]].T),
            d1T=np.ascontiguousarray(inputs["ed"][loc[1]].T),
            sgT=np.ascontiguousarray(inputs["sg"].T),
            suT=np.ascontiguousarray(inputs["su"].T),
            sdT=np.ascontiguousarray(inputs["sd"].T),
        )
        import ml_dtypes
        bfk = {"wqT", "wkT", "wvT", "wdT", "g0T", "u0T", "d0T", "g1T", "u1T",
               "d1T", "sgT", "suT", "sdT"}
        maps.append({k: (np.asarray(v, ml_dtypes.bfloat16) if k in bfk
                         else np.asarray(v, np.float32)) for k, v in m.items()})
    return maps


def kernel(**inputs):
    nc = _build()
    maps = _prep(inputs)
    res = run_bass_kernel_spmd(nc, maps, list(range(NCORES)),
                               **_BUILT.get("runkw", {}))
    _BUILT["res"] = res
    out = np.zeros((T, H), np.float32)
    for c in range(NCORES):
        r = res.results[c]
        out += r["routed"]
        out[c * TL:(c + 1) * TL] += r["own"] + r["xout"].T
    return out.reshape(B, S, H)



# revision 20
# speedup vs baseline: 1.6342x; 1.6342x over previous
"""LLaDA2 MoE decoder layer on 8 TRN2 NeuronCores.

Token-sharded attention (each core: all 16 heads for its 128 tokens, kv
projection replicated), one AllGather of post-attention normed hidden
(transposed layout), expert-parallel dense MoE (2 experts/core, gate
columns permuted per-core so local experts are columns 0,1), shared
expert token-sharded. Host sums the 8 partial outputs.
"""
import numpy as np
import concourse.bass as bass
import concourse.bacc as bacc
import concourse.mybir as mybir
import concourse.tile as tile
from concourse.bass_utils import run_bass_kernel_spmd

AF = mybir.ActivationFunctionType
ALU = mybir.AluOpType
F32 = mybir.dt.float32
F32R = mybir.dt.float32r
BF16 = mybir.dt.bfloat16

B, S, H = 1, 1024, 2048
NH, HD, NKV, ROT = 16, 128, 4, 64
E, TOPK, G = 16, 4, 2
MI = 1024
T = S
NCORES = 8
TL = T // NCORES
SCAL = HD ** -0.5
EPS = 1e-6
NDH = H // 128
NMI = MI // 128

_BUILT = {}


def _spec():
    return [
        ("hidT", [H, T], BF16), ("hidTl", [H, TL], F32),
        ("onec", [128, 1], F32R),
        ("cosl", [ROT, TL], F32), ("sinl", [ROT, TL], F32),
        ("cosf", [ROT, T], F32), ("sinf", [ROT, T], F32),
        ("qln", [HD, 1], F32), ("kln", [HD, 1], F32),
        ("ln1c", [H, 1], F32), ("ln2c", [H, 1], F32),
        ("wqT", [H, NH * HD], BF16), ("wkT", [H, NKV * HD], BF16),
        ("wvT", [H, NKV * HD], BF16), ("wdT", [NH * HD, H], BF16),
        ("gT", [H, E], BF16), ("eb", [1, E], F32),
        ("g0T", [H, MI], BF16), ("u0T", [H, MI], BF16), ("d0T", [MI, H], BF16),
        ("g1T", [H, MI], BF16), ("u1T", [H, MI], BF16), ("d1T", [MI, H], BF16),
        ("sgT", [H, MI], BF16), ("suT", [H, MI], BF16), ("sdT", [MI, H], BF16),
    ]


def _build():
    if "nc" in _BUILT:
        return _BUILT["nc"]
    nc = bacc.Bacc("TRN2", target_bir_lowering=False, debug=False,
                   num_devices=NCORES)
    I = {}
    for name, shp, dt in _spec():
        I[name] = nc.dram_tensor(name, shp, dt, kind="ExternalInput")
    routed = nc.dram_tensor("routed", [T, H], F32, kind="ExternalOutput")
    own = nc.dram_tensor("own", [TL, H], F32, kind="ExternalOutput")
    xout = nc.dram_tensor("xout", [H, TL], F32, kind="ExternalOutput")

    with tile.TileContext(nc) as tc, \
         tc.tile_pool(name="cst", bufs=1) as cst, \
         tc.tile_pool(name="big", bufs=16) as big, \
         tc.tile_pool(name="kro", bufs=4) as krop, \
         tc.tile_pool(name="vp", bufs=8) as vp, \
         tc.tile_pool(name="otp", bufs=16) as otp, \
         tc.tile_pool(name="agl", bufs=16) as agl, \
         tc.tile_pool(name="a12", bufs=16) as a12p, \
         tc.tile_pool(name="wrk", bufs=2) as wrk, \
         tc.tile_pool(name="w128", bufs=8) as w128, \
         tc.tile_pool(name="w512", bufs=2) as w512, \
         tc.tile_pool(name="yp", bufs=2) as yp, \
         tc.tile_pool(name="psA", bufs=4, space="PSUM") as psA, \
         tc.tile_pool(name="psB", bufs=4, space="PSUM") as psB, \
         tc.tile_pool(name="dram", bufs=1, space="DRAM") as dpool:

        ones = cst.tile([128, 1], F32R, tag="ones")
        nc.sync.dma_start(out=ones[:, :], in_=I["onec"][:, :])
        ones_bf = cst.tile([128, 1], BF16, tag="ones_bf")
        nc.vector.memset(ones_bf[:, :], 1.0)
        epsA = cst.tile([128, 1], F32, tag="epsA")
        nc.vector.memset(epsA[:, :], EPS)
        invH = cst.tile([128, 1], F32, tag="invH")
        nc.vector.memset(invH[:, :], 1.0 / H)
        invHD = cst.tile([128, 1], F32, tag="invHD")
        nc.vector.memset(invHD[:, :], 1.0 / HD)
        scalA = cst.tile([128, 1], F32, tag="scalA")
        nc.vector.memset(scalA[:, :], SCAL)

        def cload(name, shp, key):
            t_ = cst.tile(shp, F32, tag=key)
            nc.sync.dma_start(out=t_[:, :], in_=I[name][:, :])
            return t_
        qln = cload("qln", [HD, 1], "qln")
        kln = cload("kln", [HD, 1], "kln")
        cosl = cload("cosl", [ROT, TL], "cosl")
        sinl = cload("sinl", [ROT, TL], "sinl")
        cosf = cload("cosf", [ROT, T], "cosf")
        sinf = cload("sinf", [ROT, T], "sinf")
        ebbc = cst.tile([128, E], F32, tag="ebbc")
        nc.sync.dma_start(out=ebbc[:, :],
                          in_=I["eb"][0:1, :].partition_broadcast(128))

        def bcast(row_ap, n, tag, out_tile):
            d_ = dpool.tile([1, n], F32, tag=tag + "_d", bufs=2,
                            name=tag + "_d")
            nc.sync.dma_start(out=d_[0:1, :], in_=row_ap)
            nc.sync.dma_start(out=out_tile[:, :],
                              in_=d_[0:1, :].partition_broadcast(128))

        # ---- r_row over H from hidT (streamed) ----
        ssq = [psB.tile([1, 512], F32, tag="psB", name=f"ssq{c}")
               for c in range(2)]
        for i in range(NDH):
            ht = wrk.tile([128, T], BF16, tag="hidT", bufs=3)
            eng = nc.sync if i % 2 == 0 else nc.scalar
            eng.dma_start(out=ht[:, :], in_=I["hidT"][i * 128:(i + 1) * 128, :])
            sq = wrk.tile([128, T], BF16, tag="sq", bufs=2)
            nc.scalar.activation(sq[:, :], ht[:, :], AF.Square)
            for c in range(2):
                nc.tensor.matmul(ssq[c][:, :], ones_bf[:, :],
                                 sq[:, c * 512:(c + 1) * 512],
                                 start=(i == 0), stop=(i == NDH - 1))
        r_row = wrk.tile([1, T], F32, tag="rrow", bufs=1)
        rsq = wrk.tile([1, T], F32, tag="rsq", bufs=1)
        for c in range(2):
            nc.scalar.activation(rsq[0:1, c * 512:(c + 1) * 512], ssq[c][:, :],
                                 AF.Sqrt, bias=epsA[0:1, 0:1],
                                 scale=invH[0:1, 0:1])
        nc.vector.reciprocal(r_row[0:1, :], rsq[0:1, :])
        rbc = wrk.tile([128, T], F32, tag="rbc", bufs=1)
        bcast(r_row[0:1, :], T, "rbc", rbc)

        # ---- xnT = hidT * ln1 * r (transposed normed hidden, f32r) ----
        xnT = []
        for i in range(NDH):
            ht = wrk.tile([128, T], BF16, tag="hidT", bufs=3)
            eng = nc.scalar if i % 2 == 0 else nc.sync
            eng.dma_start(out=ht[:, :], in_=I["hidT"][i * 128:(i + 1) * 128, :])
            lnc = wrk.tile([128, 1], F32, tag="lnc", bufs=2)
            nc.sync.dma_start(out=lnc[:, :], in_=I["ln1c"][i * 128:(i + 1) * 128, :])
            xt = big.tile([128, T], BF16, tag="big")
            nc.vector.scalar_tensor_tensor(xt[:, :], ht[:, :], lnc[:, 0:1],
                                           rbc[:, :], ALU.mult, ALU.mult)
            xnT.append(xt)

        # ---- local-token normed tiles for q projection ----
        ssl = psB.tile([1, TL], F32, tag="psB", name="ssl")
        for i in range(NDH):
            htl = wrk.tile([128, TL], F32, tag="htl", bufs=2)
            nc.sync.dma_start(out=htl[:, :], in_=I["hidTl"][i * 128:(i + 1) * 128, :])
            sql = wrk.tile([128, TL], F32R, tag="sql", bufs=2)
            nc.scalar.activation(sql[:, :], htl[:, :], AF.Square)
            nc.tensor.matmul(ssl[:, :], ones[:, :], sql[:, :],
                             start=(i == 0), stop=(i == NDH - 1))
        rls = wrk.tile([1, TL], F32, tag="rls", bufs=1)
        nc.scalar.activation(rls[0:1, :], ssl[:, :], AF.Sqrt,
                             bias=epsA[0:1, 0:1], scale=invH[0:1, 0:1])
        rl = wrk.tile([1, TL], F32, tag="rl", bufs=1)
        nc.vector.reciprocal(rl[0:1, :], rls[0:1, :])
        rlb = wrk.tile([128, TL], F32, tag="rlb", bufs=1)
        bcast(rl[0:1, :], TL, "rlb", rlb)
        xnTl = []
        for i in range(NDH):
            htl2 = wrk.tile([128, TL], F32, tag="htl", bufs=2)
            nc.sync.dma_start(out=htl2[:, :],
                              in_=I["hidTl"][i * 128:(i + 1) * 128, :])
            lnc2 = wrk.tile([128, 1], F32, tag="lnc", bufs=2)
            nc.sync.dma_start(out=lnc2[:, :], in_=I["ln1c"][i * 128:(i + 1) * 128, :])
            xl = wrk.tile([128, TL], BF16, tag="xnTl", bufs=16)
            nc.vector.scalar_tensor_tensor(xl[:, :], htl2[:, :], lnc2[:, 0:1],
                                           rlb[:, :], ALU.mult, ALU.mult)
            xnTl.append(xl)

        def rms_cols(ps, n, lnw, out_ap):
            """out = ps * lnw * rsqrt(mean_part(ps^2)+eps); ps [128,n] psum."""
            sqk = wrk.tile([128, n], F32R, tag="sqk", bufs=1)
            nc.scalar.activation(sqk[:, :], ps[:, :], AF.Square)
            ssk = psB.tile([1, n], F32, tag="psB")
            nc.tensor.matmul(ssk[:, :], ones[:, :], sqk[:, :], start=True, stop=True)
            rks = wrk.tile([1, n], F32, tag="rks", bufs=1)
            nc.scalar.activation(rks[0:1, :], ssk[:, :], AF.Sqrt,
                                 bias=epsA[0:1, 0:1], scale=invHD[0:1, 0:1])
            rk = wrk.tile([1, n], F32, tag="rk", bufs=1)
            nc.vector.reciprocal(rk[0:1, :], rks[0:1, :])
            rkb = wrk.tile([128, n], F32, tag="rkb", bufs=1)
            bcast(rk[0:1, :], n, "rkb", rkb)
            nc.vector.scalar_tensor_tensor(out_ap, ps[:, :], lnw[:, 0:1],
                                           rkb[:, :], ALU.mult, ALU.mult)

        def rope(dst, src, cos_t, sin_t, n):
            """dst[0:128,n] f32r from src f32: rows 0..63 roped, 64..127 copy."""
            nc.vector.tensor_copy(dst[ROT:HD, :], src[ROT:HD, :])
            sh = wrk.tile([ROT, n], F32, tag="sh", bufs=1)
            nc.sync.dma_start(out=sh[0:32, :], in_=src[32:64, :])
            nc.sync.dma_start(out=sh[32:64, :], in_=src[0:32, :])
            tm = wrk.tile([ROT, n], F32, tag="tm", bufs=1)
            nc.vector.tensor_tensor(tm[:, :], src[0:ROT, :], cos_t[:, :], ALU.mult)
            tm2 = wrk.tile([ROT, n], F32, tag="tm2", bufs=1)
            nc.vector.tensor_tensor(tm2[:, :], sh[:, :], sin_t[:, :], ALU.mult)
            nc.vector.tensor_tensor(dst[0:ROT, :], tm[:, :], tm2[:, :], ALU.add)

        # ---- k heads: project (batched loads), then rms+rope per g ----
        kro = [krop.tile([128, T], BF16, tag="kro", name=f"kro{g}")
               for g in range(NKV)]
        for c in range(2):
            sl = slice(c * 512, (c + 1) * 512)
            psK = [psA.tile([128, 512], F32, tag="psA", name=f"psK{c}{g}")
                   for g in range(NKV)]
            for i in range(NDH):
                wkt = w512.tile([128, 512], BF16, tag="wkv", bufs=4)
                eng = nc.sync if i % 2 == 0 else nc.scalar
                eng.dma_start(out=wkt[:, :],
                              in_=I["wkT"][i * 128:(i + 1) * 128, :])
                for g in range(NKV):
                    nc.tensor.matmul(psK[g][:, :],
                                     wkt[:, g * 128:(g + 1) * 128],
                                     xnT[i][:, sl],
                                     start=(i == 0), stop=(i == NDH - 1))
            for g in range(NKV):
                kf = wrk.tile([128, 512], F32, tag="kf", bufs=2)
                rms_cols(psK[g], 512, kln, kf[:, :])
                rope(kro[g][:, sl], kf, cosf[:, sl], sinf[:, sl], 512)

        # ---- v token-major [t-tile, 512] ----
        vsb = [None] * 8
        for jo in range(2):
            psV = [psB.tile([128, 512], F32, tag="psB", name=f"psV{jo}{k}")
                   for k in range(4)]
            for i in range(NDH):
                wvt = w512.tile([128, 512], BF16, tag="wkv", bufs=4)
                eng = nc.scalar if i % 2 == 0 else nc.sync
                eng.dma_start(out=wvt[:, :],
                              in_=I["wvT"][i * 128:(i + 1) * 128, :])
                for jj in range(4):
                    j = jo * 4 + jj
                    nc.tensor.matmul(psV[jj][:, :],
                                     xnT[i][:, j * 128:(j + 1) * 128],
                                     wvt[:, :], start=(i == 0),
                                     stop=(i == NDH - 1))
            for jj in range(4):
                vt = vp.tile([128, 512], BF16, tag="vp")
                nc.vector.tensor_copy(vt[:, :], psV[jj][:, :])
                vsb[jo * 4 + jj] = vt

        # ---- per q-head: project(local), rms, rope, scores, probs, oT ----
        oT = []
        for h in range(NH):
            g = h // (NH // NKV)
            ps = psB.tile([128, TL], F32, tag="psB")
            for i in range(NDH):
                wt = w128.tile([128, 128], BF16, tag="w128")
                eng = nc.sync if (h * NDH + i) % 2 == 0 else nc.scalar
                eng.dma_start(
                    out=wt[:, :],
                    in_=I["wqT"][i * 128:(i + 1) * 128, h * 128:(h + 1) * 128])
                nc.tensor.matmul(ps[:, :], wt[:, :], xnTl[i][:, :],
                                 start=(i == 0), stop=(i == NDH - 1))
            qf = wrk.tile([128, TL], F32, tag="qf", bufs=2)
            rms_cols(ps, TL, qln, qf[:, :])
            qr = wrk.tile([128, TL], BF16, tag="qr", bufs=2)
            rope(qr, qf, cosl, sinl, TL)
            # scores^T tiles [tk 128, tq 128]; probs = exp(s*SCAL); oT accum
            pso = psB.tile([128, TL], F32, tag="psB")
            psz = psB.tile([1, TL], F32, tag="psB")
            for tk in range(8):
                sps = psB.tile([128, TL], F32, tag="psB")
                nc.tensor.matmul(sps[:, :], kro[g][:, tk * 128:(tk + 1) * 128],
                                 qr[:, :], start=True, stop=True)
                pr = wrk.tile([128, TL], BF16, tag="pr", bufs=3)
                nc.scalar.activation(pr[:, :], sps[:, :], AF.Exp,
                                     scale=scalA[:, 0:1])
                nc.tensor.matmul(pso[:, :], vsb[tk][:, g * 128:(g + 1) * 128],
                                 pr[:, :], start=(tk == 0), stop=(tk == 7))
                nc.tensor.matmul(psz[:, :], ones_bf[:, :], pr[:, :],
                                 start=(tk == 0), stop=(tk == 7))
            zr = wrk.tile([1, TL], F32, tag="zr", bufs=2)
            nc.vector.reciprocal(zr[0:1, :], psz[:, :])
            zbc = wrk.tile([128, TL], F32, tag="zbc", bufs=2)
            bcast(zr[0:1, :], TL, "zbc", zbc)
            ot = otp.tile([128, TL], BF16, tag="oT")
            nc.vector.tensor_tensor(ot[:, :], pso[:, :], zbc[:, :], ALU.mult)
            oT.append(ot)

        # ---- attn_outT + residual -> xT; rms -> hT (f32r) + xout/ag_in ----
        ag_in = dpool.tile([H, TL], F32R, tag="agin")
        ag_out = dpool.tile([NCORES * H, TL], F32R, tag="agout",
                            addr_space="Shared")
        hT_l = [None] * NDH
        for io in range(4):
            psX = [psB.tile([128, TL], F32, tag="psB", name=f"psX{io}_{k}")
                   for k in range(4)]
            for d in range(NH):
                wt = w512.tile([128, 512], BF16, tag="wd512", bufs=4)
                eng = nc.sync if d % 2 == 0 else nc.scalar
                eng.dma_start(
                    out=wt[:, :],
                    in_=I["wdT"][d * 128:(d + 1) * 128, io * 512:(io + 1) * 512])
                for di in range(4):
                    nc.tensor.matmul(psX[di][:, :],
                                     wt[:, di * 128:(di + 1) * 128],
                                     oT[d][:, :],
                                     start=(d == 0), stop=(d == NH - 1))
            for di in range(4):
                i = io * 4 + di
                hl = wrk.tile([128, TL], F32, tag="hl", bufs=2)
                nc.sync.dma_start(out=hl[:, :],
                                  in_=I["hidTl"][i * 128:(i + 1) * 128, :])
                xt = agl.tile([128, TL], F32, tag="xT")
                nc.vector.tensor_tensor(xt[:, :], psX[di][:, :], hl[:, :],
                                        ALU.add)
                nc.sync.dma_start(out=xout[i * 128:(i + 1) * 128, :],
                                  in_=xt[:, :])
                hT_l[i] = xt
        # second rms (over H, partition dim) via ones-matmul on squares
        ss2 = psB.tile([1, TL], F32, tag="psB")
        sq2t = []
        for i in range(NDH):
            s2 = wrk.tile([128, TL], BF16, tag="s2", bufs=16)
            nc.scalar.activation(s2[:, :], hT_l[i][:, :], AF.Square)
            sq2t.append(s2)
        for i in range(NDH):
            nc.tensor.matmul(ss2[:, :], ones_bf[:, :], sq2t[i][:, :],
                             start=(i == 0), stop=(i == NDH - 1))
        r2s = wrk.tile([1, TL], F32, tag="r2s", bufs=1)
        nc.scalar.activation(r2s[0:1, :], ss2[:, :], AF.Sqrt,
                             bias=epsA[0:1, 0:1], scale=invH[0:1, 0:1])
        r2 = wrk.tile([1, TL], F32, tag="r2", bufs=1)
        nc.vector.reciprocal(r2[0:1, :], r2s[0:1, :])
        r2b = wrk.tile([128, TL], F32, tag="r2b", bufs=1)
        bcast(r2[0:1, :], TL, "r2b", r2b)
        hTt = []
        for i in range(NDH):
            ln2 = wrk.tile([128, 1], F32, tag="ln2", bufs=2)
            nc.sync.dma_start(out=ln2[:, :], in_=I["ln2c"][i * 128:(i + 1) * 128, :])
            ht = agl.tile([128, TL], F32R, tag="hTl")
            nc.vector.scalar_tensor_tensor(ht[:, :], hT_l[i][:, :], ln2[:, 0:1],
                                           r2b[:, :], ALU.mult, ALU.mult)
            nc.sync.dma_start(out=ag_in[i * 128:(i + 1) * 128, :], in_=ht[:, :])
            hTt.append(ht)

        nc.gpsimd.collective_compute(
            "AllGather", ALU.bypass, ins=[ag_in], outs=[ag_out],
            replica_groups=[list(range(NCORES))])

        # ---- load gathered hT [2048, 1024] f32r into big pool ----
        agv = ag_out.rearrange("(b d) t -> d b t", b=NCORES)
        hsb = []
        for i in range(NDH):
            t_ = big.tile([128, T], BF16, tag="big")
            nc.gpsimd.dma_start(out=t_[:, :], in_=agv[i * 128:(i + 1) * 128, :, :])
            hsb.append(t_)
        hbf = []
        for i in range(NDH):
            hb = agl.tile([128, TL], BF16, tag="hbf")
            nc.vector.tensor_copy(hb[:, :], hTt[i][:, :])
            hbf.append(hb)
        _BUILT["ctx"] = dict(nc=nc, tc=tc, I=I, routed=routed, own=own,
                             hsb=hsb, hTt=hTt, ones=ones, ebbc=ebbc,
                             pools=dict(cst=cst, a12p=a12p, wrk=wrk, w128=w128,
                                        w512=w512, yp=yp, psA=psA, psB=psB))
        _moe(nc, tc, I, routed, own, hsb, hbf, agv, ebbc,
             a12p, wrk, w128, w512, yp, psA, psB)
    nc.compile()
    _BUILT["nc"] = nc
    return nc


def _moe(nc, tc, I, routed, own, hsb, hbf, agv, ebbc, a12p, wrk, w128, w512,
         yp, psA, psB):
    # ---- routing (replicated, all tokens): we [128,16] f32 per t-tile ----
    gts = []
    for i in range(NDH):
        gt = wrk.tile([128, E], BF16, tag="gt", bufs=16)
        nc.sync.dma_start(out=gt[:, :], in_=I["gT"][i * 128:(i + 1) * 128, :])
        gts.append(gt)
    we_sb = []
    for j in range(8):
        pl = psB.tile([128, E], F32, tag="psB")
        for i in range(NDH):
            nc.tensor.matmul(pl[:, :], hsb[i][:, j * 128:(j + 1) * 128],
                             gts[i][:, :],
                             start=(i == 0), stop=(i == NDH - 1))
        s = wrk.tile([128, E], F32, tag="rs", bufs=2)
        nc.scalar.activation(s[:, :], pl[:, :], AF.Sigmoid)
        sfr = wrk.tile([128, E], F32, tag="sfr", bufs=2)
        nc.vector.tensor_tensor(sfr[:, :], s[:, :], ebbc[:, :], ALU.add)
        msk = wrk.tile([128, E], F32, tag="msk", bufs=2)
        m1 = wrk.tile([128, 2], F32, tag="m1", bufs=2)
        m2 = wrk.tile([128, 2], F32, tag="m2", bufs=2)
        tmp8 = wrk.tile([128, 8], F32, tag="tmp8", bufs=2)
        for g in range(2):
            hv = sfr[:, g * 8:(g + 1) * 8]
            nc.vector.tensor_reduce(m1[:, g:g + 1], hv, mybir.AxisListType.X,
                                    ALU.max)
            eq = wrk.tile([128, 8], F32, tag="eq", bufs=2)
            nc.vector.tensor_scalar(eq[:, :], hv, m1[:, g:g + 1], None,
                                    ALU.is_equal)
            nc.vector.scalar_tensor_tensor(tmp8[:, :], eq[:, :], -1e30,
                                           hv, ALU.mult, ALU.add)
            nc.vector.tensor_reduce(m2[:, g:g + 1], tmp8[:, :],
                                    mybir.AxisListType.X, ALU.max)
        gs = wrk.tile([128, 2], F32, tag="gs", bufs=2)
        nc.vector.tensor_tensor(gs[:, :], m1[:, :], m2[:, :], ALU.add)
        gd = wrk.tile([128, 1], F32, tag="gd", bufs=2)
        nc.vector.tensor_tensor(gd[:, :], gs[:, 0:1], gs[:, 1:2], ALU.subtract)
        ka = wrk.tile([128, 2], F32, tag="ka", bufs=2)
        nc.vector.tensor_scalar(ka[:, 0:1], gd[:, :], 0.0, None, ALU.is_ge)
        nc.vector.tensor_scalar(ka[:, 1:2], ka[:, 0:1], -1.0, 1.0,
                                ALU.mult, ALU.add)
        for g in range(2):
            nc.vector.tensor_scalar(msk[:, g * 8:(g + 1) * 8],
                                    sfr[:, g * 8:(g + 1) * 8],
                                    ka[:, g:g + 1], None, ALU.mult)
        # 4th-largest threshold of msk
        w0 = wrk.tile([128, E], F32, tag="w0", bufs=2)
        nc.vector.tensor_copy(w0[:, :], msk[:, :])
        tau = wrk.tile([128, 1], F32, tag="tau", bufs=2)
        lt = wrk.tile([128, E], F32, tag="lt", bufs=2)
        for it in range(3):
            nc.vector.tensor_reduce(tau[:, :], w0[:, :], mybir.AxisListType.X,
                                    ALU.max)
            nc.vector.tensor_scalar(lt[:, :], w0[:, :], tau[:, 0:1], None,
                                    ALU.is_lt)
            nc.vector.tensor_tensor(w0[:, :], w0[:, :], lt[:, :], ALU.mult)
        nc.vector.tensor_reduce(tau[:, :], w0[:, :], mybir.AxisListType.X,
                                ALU.max)
        sel = wrk.tile([128, E], F32, tag="sel", bufs=2)
        nc.vector.tensor_scalar(sel[:, :], msk[:, :], tau[:, 0:1], None,
                                ALU.is_ge)
        wsel = wrk.tile([128, E], F32, tag="wsel", bufs=2)
        nc.vector.tensor_tensor(wsel[:, :], s[:, :], sel[:, :], ALU.mult)
        dn = wrk.tile([128, 1], F32, tag="dn", bufs=2)
        nc.vector.tensor_reduce(dn[:, :], wsel[:, :], mybir.AxisListType.X,
                                ALU.add)
        nc.vector.tensor_scalar(dn[:, :], dn[:, :], 1e-20, None, ALU.add)
        rc = wrk.tile([128, 1], F32, tag="rc", bufs=2)
        nc.vector.reciprocal(rc[:, :], dn[:, :])
        we = wrk.tile([128, E], F32, tag="we", bufs=16)
        nc.vector.tensor_scalar(we[:, :], wsel[:, :], rc[:, 0:1], None,
                                ALU.mult)
        we_sb.append(we)

    # ---- routed experts: dense over all tokens, 2 local experts ----
    for tc_i in range(2):          # token chunk of 512
        tsl = slice(tc_i * 512, (tc_i + 1) * 512)
        a12 = {}
        for e, (gn, un) in enumerate((("g0T", "u0T"), ("g1T", "u1T"))):
            for mo in range(2):
                psG = [psA.tile([128, 512], F32, tag="psA",
                                name=f"psG{tc_i}{e}{mo}{k}") for k in range(4)]
                psU = [psB.tile([128, 512], F32, tag="psB",
                                name=f"psU{tc_i}{e}{mo}{k}") for k in range(4)]
                for i in range(NDH):
                    rh = hsb[i][:, tsl]
                    wg = w512.tile([128, 512], BF16, tag="wgu", bufs=6)
                    nc.sync.dma_start(
                        out=wg[:, :],
                        in_=I[gn][i * 128:(i + 1) * 128,
                                  mo * 512:(mo + 1) * 512])
                    wu = w512.tile([128, 512], BF16, tag="wgu", bufs=6)
                    nc.scalar.dma_start(
                        out=wu[:, :],
                        in_=I[un][i * 128:(i + 1) * 128,
                                  mo * 512:(mo + 1) * 512])
                    for dmi in range(4):
                        csl = slice(dmi * 128, (dmi + 1) * 128)
                        nc.tensor.matmul(psG[dmi][:, :], wg[:, csl], rh,
                                         start=(i == 0), stop=(i == NDH - 1))
                        nc.tensor.matmul(psU[dmi][:, :], wu[:, csl], rh,
                                         start=(i == 0), stop=(i == NDH - 1))
                for dmi in range(4):
                    mi = mo * 4 + dmi
                    sg = wrk.tile([128, 512], F32, tag="sg", bufs=3)
                    nc.scalar.activation(sg[:, :], psG[dmi][:, :], AF.Silu)
                    at = a12p.tile([128, 512], BF16, tag="a12")
                    nc.vector.tensor_tensor(at[:, :], sg[:, :],
                                            psU[dmi][:, :], ALU.mult)
                    a12[(e, mi)] = at
        for ho in range(4):
            p0 = [psA.tile([128, 512], F32, tag="psA", name=f"p0_{tc_i}{ho}{k}")
                  for k in range(4)]
            p1 = [psB.tile([128, 512], F32, tag="psB", name=f"p1_{tc_i}{ho}{k}")
                  for k in range(4)]
            for mi in range(NMI):
                w0_ = w512.tile([128, 512], BF16, tag="edp0", bufs=3)
                nc.sync.dma_start(
                    out=w0_[:, :],
                    in_=I["d0T"][mi * 128:(mi + 1) * 128, ho * 512:(ho + 1) * 512])
                w1_ = w512.tile([128, 512], BF16, tag="edp1", bufs=3)
                nc.scalar.dma_start(
                    out=w1_[:, :],
                    in_=I["d1T"][mi * 128:(mi + 1) * 128, ho * 512:(ho + 1) * 512])
                for ts in range(4):
                    cs = slice(ts * 128, (ts + 1) * 128)
                    nc.tensor.matmul(p0[ts][:, :], a12[(0, mi)][:, cs],
                                     w0_[:, :],
                                     start=(mi == 0), stop=(mi == NMI - 1))
                    nc.tensor.matmul(p1[ts][:, :], a12[(1, mi)][:, cs],
                                     w1_[:, :],
                                     start=(mi == 0), stop=(mi == NMI - 1))
            for ts in range(4):
                jj = tc_i * 4 + ts
                y = yp.tile([128, 512], F32, tag="y")
                nc.vector.tensor_scalar(y[:, :], p0[ts][:, :],
                                        we_sb[jj][:, 0:1], None, ALU.mult)
                nc.vector.scalar_tensor_tensor(y[:, :], p1[ts][:, :],
                                               we_sb[jj][:, 1:2], y[:, :],
                                               ALU.mult, ALU.add)
                nc.sync.dma_start(
                    out=routed[jj * 128:(jj + 1) * 128, ho * 512:(ho + 1) * 512],
                    in_=y[:, :])

    # ---- shared expert on local 128 tokens ----
    a12s = [None] * NMI
    for mo in range(2):
        psSG = [psA.tile([128, TL], F32, tag="psA", name=f"psSG{mo}{k}")
                for k in range(4)]
        psSU = [psB.tile([128, TL], F32, tag="psB", name=f"psSU{mo}{k}")
                for k in range(4)]
        for i in range(NDH):
            wg = w512.tile([128, 512], BF16, tag="wgu", bufs=6)
            nc.sync.dma_start(
                out=wg[:, :],
                in_=I["sgT"][i * 128:(i + 1) * 128, mo * 512:(mo + 1) * 512])
            wu = w512.tile([128, 512], BF16, tag="wgu", bufs=6)
            nc.scalar.dma_start(
                out=wu[:, :],
                in_=I["suT"][i * 128:(i + 1) * 128, mo * 512:(mo + 1) * 512])
            for dmi in range(4):
                csl = slice(dmi * 128, (dmi + 1) * 128)
                nc.tensor.matmul(psSG[dmi][:, :], wg[:, csl], hbf[i][:, :],
                                 start=(i == 0), stop=(i == NDH - 1))
                nc.tensor.matmul(psSU[dmi][:, :], wu[:, csl], hbf[i][:, :],
                                 start=(i == 0), stop=(i == NDH - 1))
        for dmi in range(4):
            mi = mo * 4 + dmi
            sg = wrk.tile([128, TL], F32, tag="sgs", bufs=2)
            nc.scalar.activation(sg[:, :], psSG[dmi][:, :], AF.Silu)
            at = a12p.tile([128, TL], BF16, tag="a12s", bufs=8)
            nc.vector.tensor_tensor(at[:, :], sg[:, :], psSU[dmi][:, :],
                                    ALU.mult)
            a12s[mi] = at
    for ho in range(4):
        ps = psA.tile([128, 512], F32, tag="psA")
        for mi in range(NMI):
            wt = w512.tile([128, 512], BF16, tag="sdw", bufs=4)
            eng = nc.sync if mi % 2 == 0 else nc.scalar
            eng.dma_start(
                out=wt[:, :],
                in_=I["sdT"][mi * 128:(mi + 1) * 128, ho * 512:(ho + 1) * 512])
            nc.tensor.matmul(ps[:, :], a12s[mi][:, :], wt[:, :],
                             start=(mi == 0), stop=(mi == NMI - 1))
        y = yp.tile([128, 512], F32, tag="y")
        nc.vector.tensor_copy(y[:, :], ps[:, :])
        nc.sync.dma_start(out=own[:, ho * 512:(ho + 1) * 512], in_=y[:, :])


def _prep(inputs):
    """Per-core input dicts from full inputs."""
    hs = np.ascontiguousarray(inputs["hidden_states"][0])      # [T, H]
    hidT = np.ascontiguousarray(hs.T)                          # [H, T]
    cos = np.ascontiguousarray(inputs["cos"][0].T)             # [ROT, T]
    sin = inputs["sin"][0].T                                   # [ROT, T]
    sgn = np.ones((ROT, 1), np.float32)
    sgn[:32] = -1.0
    sins = np.ascontiguousarray(sin * sgn)
    wqkv = inputs["w_qkv"]
    wqT = np.ascontiguousarray(wqkv[:NH * HD].T)
    wkT = np.ascontiguousarray(wqkv[NH * HD:NH * HD + NKV * HD].T)
    wvT = np.ascontiguousarray(wqkv[NH * HD + NKV * HD:].T)
    wdT = np.ascontiguousarray(inputs["w_dense"].T)
    maps = []
    for c in range(NCORES):
        glo = c // 4
        loc = [2 * c, 2 * c + 1]
        grp = [glo * 8 + k for k in range(8)]
        rest = [e for e in grp if e not in loc]
        other = [(1 - glo) * 8 + k for k in range(8)]
        perm = loc + rest + other
        m = dict(
            hidT=hidT, hidTl=np.ascontiguousarray(hidT[:, c * TL:(c + 1) * TL]),
            onec=np.ones((128, 1), np.float32),
            cosl=np.ascontiguousarray(cos[:, c * TL:(c + 1) * TL]),
            sinl=np.ascontiguousarray(sins[:, c * TL:(c + 1) * TL]),
            cosf=cos, sinf=sins,
            qln=np.ascontiguousarray(inputs["q_ln_w"][:, None]),
            kln=np.ascontiguousarray(inputs["k_ln_w"][:, None]),
            ln1c=np.ascontiguousarray(inputs["ln1_w"][:, None]),
            ln2c=np.ascontiguousarray(inputs["ln2_w"][:, None]),
            wqT=wqT, wkT=wkT, wvT=wvT, wdT=wdT,
            gT=np.ascontiguousarray(inputs["gate_w"][perm].T),
            eb=np.ascontiguousarray(inputs["expert_bias"][perm][None, :]),
            g0T=np.ascontiguousarray(inputs["eg"][loc[0]].T),
            u0T=np.ascontiguousarray(inputs["eu"][loc[0]].T),
            d0T=np.ascontiguousarray(inputs["ed"][loc[0]].T),
            g1T=np.ascontiguousarray(inputs["eg"][loc[1]].T),
            u1T=np.ascontiguousarray(inputs["eu"][loc[1]].T),
            d1T=np.ascontiguousarray(inputs["ed"][loc[1]].T),
            sgT=np.ascontiguousarray(inputs["sg"].T),
            suT=np.ascontiguousarray(inputs["su"].T),
            sdT=np.ascontiguousarray(inputs["sd"].T),
        )
        import ml_dtypes
        bfk = {"wqT", "wkT", "wvT", "wdT", "g0T", "u0T", "d0T", "g1T", "u1T",
               "d1T", "sgT", "suT", "sdT", "gT", "hidT"}
        maps.append({k: (np.asarray(v, ml_dtypes.bfloat16) if k in bfk
                         else np.asarray(v, np.float32)) for k, v in m.items()})
    return maps


def kernel(**inputs):
    nc = _build()
    maps = _prep(inputs)
    res = run_bass_kernel_spmd(nc, maps, list(range(NCORES)),
                               **_BUILT.get("runkw", {}))
    _BUILT["res"] = res
    out = np.zeros((T, H), np.float32)
    for c in range(NCORES):
        r = res.results[c]
        out += r["routed"]
        out[c * TL:(c + 1) * TL] += r["own"] + r["xout"].T
    return out.reshape(B, S, H)

